# revision 13
# baseline (speedup 1.0000x reference)
"""Trainium2 Bass kernel for nn_CLOSpreadModel (moe_routing, 8 cores).

Math (per sample n):
  out[n] = hinge(mvoc; base) + hinge(mvoc; adj[bucket_idx[n]]) + adj_b[bucket]
         + hinge(lev; idx) + hinge(wap; wap) + hinge(cpn; cpn) + bias
  where hinge(x; knots,w,b) = sum_k w_k * relu(x - knot_k) + b    (K=64)

Strategy:
  * Host: route samples by bucket (the MoE routing): stable-sort by bucket_idx,
    pad each bucket's segment to a fixed 65536-slot region, deal the slots to
    8 cores so every core gets a fixed [16 buckets x 8192 samples] layout.
    All knot/weight tables are packed host-side into small constant matrices.
  * Device (identical program on all 8 cores, 131072 samples each):
    For each 512-sample chunk, the PE builds delta tiles D[(h,k), f] =
    x_h[f] - knot_hk with tiny-contraction matmuls (rhs rows = [1, mvoc, lev,
    wap, cpn, cpn_shift]); ACT/DVE apply relu PSUM->SBUF; PE contracts the
    128 knot-rows against weight vectors, accumulating all five hinges (plus
    the summed bias via a dedicated ones-row matmul) into per-sample PSUM
    rows.  The two cpn hinges of a chunk pair share one delta tile via a
    512-shifted cpn row, saving a fifth of the PE passes.
"""

import os
import sys

import numpy as np

sys.path.insert(0, "/opt/trn_rl_repo")

N_CORES = 8
B = 16          # buckets
K = 64          # knots per hinge
SEG = 8192      # samples per (core, bucket) block
NCORE = B * SEG           # samples per core = 131072
NPAD = N_CORES * NCORE    # padded total = 1048576
F = 512                   # chunk size (one PSUM bank of fp32)

LHW = 18 * 128            # lhsT blocks: 16x A[b], B, C-pair
WS1 = LHW                 # 17 single-col weight blocks, 15 wide each
W3P = WS1 + 17 * 15       # dual-col cpn weight block, 16 wide
BIA = W3P + 16            # per-bucket bias strip on row 0, 8 wide each
CSTW = BIA + 16 * 8

_CACHE = {}


def _build_bass():
    """Build the per-core Bass module (identical for all cores)."""
    import concourse.bass as bass
    import concourse.bacc as bacc
    import concourse.mybir as mybir
    from concourse.tile import TileContext
    from contextlib import ExitStack

    f32 = mybir.dt.float32
    Relu = mybir.ActivationFunctionType.Relu

    nc = bacc.Bacc()
    xs = nc.declare_dram_parameter("xs", [6, NCORE], f32, isOutput=False)
    cst = nc.declare_dram_parameter("cst", [128, CSTW], f32, isOutput=False)
    out = nc.declare_dram_parameter("out", [NCORE], f32, isOutput=True)

    with TileContext(nc) as tc:
        with ExitStack() as ctx:
            const_pool = ctx.enter_context(tc.tile_pool(name="const", bufs=1))
            xs_pool = ctx.enter_context(tc.tile_pool(name="xsp", bufs=2))
            r_pool = ctx.enter_context(tc.tile_pool(name="rp", bufs=2))
            osb_pool = ctx.enter_context(tc.tile_pool(name="osb", bufs=2))
            d_pool = ctx.enter_context(tc.tile_pool(name="dp", bufs=2, space="PSUM"))
            o_pool = ctx.enter_context(tc.tile_pool(name="op", bufs=2, space="PSUM"))

            cst_sb = const_pool.tile([128, CSTW], f32)
            nc.sync.dma_start(out=cst_sb[:], in_=cst[:, :])
            lh = cst_sb[0:6, 0:LHW]

            def wv1(k, j):        # [128, 8], weight col k at output row j
                return cst_sb[:, WS1 + k * 15 + 7 - j: WS1 + k * 15 + 15 - j]

            def wv3(t):           # [128, 8], cpn pair cols at rows 2t, 2t+1
                return cst_sb[:, W3P + 7 - 2 * t: W3P + 15 - 2 * t]

            # startup absorber: takes the const-DMA wait on its own ldweights
            # so no later matmul needs two sync waits (walrus limit is one)
            warm = o_pool.tile([8, F], f32, tag="ob")
            nc.tensor.matmul(warm[:, 0:1], cst_sb[0:5, 0:8], cst_sb[0:5, 0:1],
                             start=True, stop=True, skip_group_check=True)

            for b in range(B):
                xs_sb = xs_pool.tile([6, SEG], f32, tag="xs")
                nc.sync.dma_start(out=xs_sb[:], in_=xs[:, b * SEG:(b + 1) * SEG])
                for g in range(SEG // (8 * F)):      # 2 groups of 8 chunks
                    ob = o_pool.tile([8, F], f32, tag="ob")
                    # summed bias for this block, via the ones row
                    c0 = g * 8
                    nc.tensor.matmul(
                        ob[:], cst_sb[0:1, BIA + b * 8: BIA + b * 8 + 8],
                        xs_sb[0:1, c0 * F:(c0 + 1) * F],
                        start=True, stop=False, skip_group_check=True)
                    for t in range(4):               # chunk pairs
                        ca = g * 8 + 2 * t
                        rhs_a = xs_sb[:, ca * F:(ca + 1) * F]
                        rhs_b = xs_sb[:, (ca + 1) * F:(ca + 2) * F]
                        d1a = d_pool.tile([128, F], f32, tag="d1")
                        nc.tensor.matmul(d1a[:], lh[:, b * 128:(b + 1) * 128],
                                         rhs_a, start=True, stop=True)
                        d1b = d_pool.tile([128, F], f32, tag="d1")
                        nc.tensor.matmul(d1b[:], lh[:, b * 128:(b + 1) * 128],
                                         rhs_b, start=True, stop=True)
                        d2a = d_pool.tile([128, F], f32, tag="d2")
                        nc.tensor.matmul(d2a[:], lh[:, 16 * 128:17 * 128],
                                         rhs_a, start=True, stop=True)
                        d2b = d_pool.tile([128, F], f32, tag="d2")
                        nc.tensor.matmul(d2b[:], lh[:, 16 * 128:17 * 128],
                                         rhs_b, start=True, stop=True)
                        d3 = d_pool.tile([128, F], f32, tag="d3")
                        nc.tensor.matmul(d3[:], lh[:, 17 * 128:18 * 128],
                                         rhs_a, start=True, stop=True)

                        r1a = r_pool.tile([128, F], f32, tag="r1")
                        nc.scalar.activation(r1a[:], d1a[:], Relu)
                        r1b = r_pool.tile([128, F], f32, tag="r1b")
                        nc.scalar.activation(r1b[:], d1b[:], Relu)
                        r2a = r_pool.tile([128, F], f32, tag="r2")
                        nc.vector.tensor_scalar_max(r2a[:], d2a[:], 0.0)
                        r2b = r_pool.tile([128, F], f32, tag="r2b")
                        nc.vector.tensor_scalar_max(r2b[:], d2b[:], 0.0)
                        r3 = r_pool.tile([128, F], f32, tag="r3")
                        if t % 2 == 0:
                            nc.vector.tensor_scalar_max(r3[:], d3[:], 0.0)
                        else:
                            nc.scalar.activation(r3[:], d3[:], Relu)

                        nc.tensor.matmul(ob[:], wv1(b, 2 * t), r1a[:],
                                         start=False, stop=False,
                                         skip_group_check=True)
                        nc.tensor.matmul(ob[:], wv1(b, 2 * t + 1), r1b[:],
                                         start=False, stop=False,
                                         skip_group_check=True)
                        nc.tensor.matmul(ob[:], wv1(16, 2 * t), r2a[:],
                                         start=False, stop=False,
                                         skip_group_check=True)
                        nc.tensor.matmul(ob[:], wv1(16, 2 * t + 1), r2b[:],
                                         start=False, stop=False,
                                         skip_group_check=True)
                        nc.tensor.matmul(ob[:], wv3(t), r3[:],
                                         start=False, stop=(t == 3),
                                         skip_group_check=True)
                    out_sb = osb_pool.tile([8, F], f32, tag="out")
                    if g % 2 == 0:
                        nc.vector.tensor_copy(out_sb[:], ob[:])
                    else:
                        nc.scalar.copy(out_sb[:], ob[:])
                    dst = out[b * SEG + g * 8 * F: b * SEG + (g + 1) * 8 * F]
                    nc.sync.dma_start(
                        out=dst.rearrange("(p f) -> p f", p=8), in_=out_sb[:]
                    )
    nc.finalize()
    return nc


def _pack_consts(base_knots, base_w, base_b, adj_knots, adj_w, adj_b,
                 idx_knots, idx_w, idx_b, wap_knots, wap_w, wap_b,
                 cpn_knots, cpn_w, cpn_b, bias):
    """Pack parameters into the single constant matrix (see _build_bass)."""
    cst = np.zeros((128, CSTW), dtype=np.float32)
    for b in range(B):
        c0 = b * 128
        cst[0, c0:c0 + 64] = -base_knots
        cst[1, c0:c0 + 64] = 1.0
        cst[0, c0 + 64:c0 + 128] = -adj_knots[b]
        cst[1, c0 + 64:c0 + 128] = 1.0
    c0 = 16 * 128
    cst[0, c0:c0 + 64] = -idx_knots
    cst[2, c0:c0 + 64] = 1.0
    cst[0, c0 + 64:c0 + 128] = -wap_knots
    cst[3, c0 + 64:c0 + 128] = 1.0
    c0 = 17 * 128          # cpn pair block: row4 = this chunk, row5 = next
    cst[0, c0:c0 + 64] = -cpn_knots
    cst[4, c0:c0 + 64] = 1.0
    cst[0, c0 + 64:c0 + 128] = -cpn_knots
    cst[5, c0 + 64:c0 + 128] = 1.0
    # single-col weight blocks, zero-padded 7 each side
    for b in range(B):
        cst[0:64, WS1 + b * 15 + 7] = base_w
        cst[64:128, WS1 + b * 15 + 7] = adj_w[b]
    cst[0:64, WS1 + 16 * 15 + 7] = idx_w
    cst[64:128, WS1 + 16 * 15 + 7] = wap_w
    # dual-col cpn block
    cst[0:64, W3P + 7] = cpn_w
    cst[64:128, W3P + 8] = cpn_w
    # bias strip (row 0 feeds the ones-row matmul)
    for b in range(B):
        tot = base_b + adj_b[b] + idx_b + wap_b + cpn_b + bias
        cst[0, BIA + b * 8: BIA + b * 8 + 8] = tot
    return cst


def _hinge_np(x, knots, w, b):
    return np.maximum(x[:, None] - knots[None, :], 0.0) @ w + b


def kernel(mvoc, bucket_idx, lev_idx, wap, cpnspread,
           base_knots, base_w, base_b,
           adj_knots, adj_w, adj_b,
           idx_knots, idx_w, idx_b,
           wap_knots, wap_w, wap_b,
           cpn_knots, cpn_w, cpn_b, bias, **_unused):
    from concourse.bass_utils import run_bass_kernel_spmd

    mvoc = np.asarray(mvoc, dtype=np.float32)
    bidx = np.asarray(bucket_idx).astype(np.int64)
    lev = np.asarray(lev_idx, dtype=np.float32)
    wapv = np.asarray(wap, dtype=np.float32)
    cpn = np.asarray(cpnspread, dtype=np.float32)
    N = mvoc.shape[0]

    params = [np.asarray(a, dtype=np.float32) for a in (
        base_knots, base_w, base_b, adj_knots, adj_w, adj_b,
        idx_knots, idx_w, idx_b, wap_knots, wap_w, wap_b,
        cpn_knots, cpn_w, cpn_b, bias)]
    (base_knots, base_w, base_b, adj_knots, adj_w, adj_b,
     idx_knots, idx_w, idx_b, wap_knots, wap_w, wap_b,
     cpn_knots, cpn_w, cpn_b, bias) = params
    adj_knots = adj_knots.reshape(B, K)
    adj_w = adj_w.reshape(B, K)
    base_b = float(base_b); idx_b = float(idx_b); wap_b = float(wap_b)
    cpn_b = float(cpn_b); bias = float(bias)

    # ---- host-side routing: stable sort by bucket, pad to fixed layout ----
    CAP = NPAD // B                       # 65536 slots per bucket
    order = np.argsort(bidx, kind="stable")
    sorted_b = bidx[order]
    counts = np.bincount(bidx, minlength=B)
    starts = np.concatenate([[0], np.cumsum(counts)[:-1]])
    within = np.arange(N, dtype=np.int64) - starts[sorted_b]
    ok = within < CAP                     # overflow guard (never for ~uniform buckets)
    gslot = sorted_b[ok] * CAP + within[ok]

    def route(x):
        xp = np.zeros(NPAD, dtype=np.float32)
        xp[gslot] = x[order][ok]
        # [B, CAP] -> [B, N_CORES, SEG] -> [N_CORES, B, SEG] -> [N_CORES, NCORE]
        return (xp.reshape(B, N_CORES, SEG).transpose(1, 0, 2)
                .reshape(N_CORES, NCORE))

    xs_all = np.ones((N_CORES, 6, NCORE), dtype=np.float32)
    xs_all[:, 1] = route(mvoc)
    xs_all[:, 2] = route(lev)
    xs_all[:, 3] = route(wapv)
    xs_all[:, 4] = route(cpn)
    xs_all[:, 5, :-F] = xs_all[:, 4, F:]   # cpn shifted left one chunk
    xs_all[:, 5, -F:] = 0.0

    cstm = _pack_consts(base_knots, base_w, base_b, adj_knots, adj_w, adj_b,
                        idx_knots, idx_w, idx_b, wap_knots, wap_w, wap_b,
                        cpn_knots, cpn_w, cpn_b, bias)

    if "nc" not in _CACHE:
        _CACHE["nc"] = _build_bass()
    nc = _CACHE["nc"]

    in_maps = [{"xs": xs_all[j], "cst": cstm} for j in range(N_CORES)]
    _CACHE["in_maps"] = in_maps
    res = run_bass_kernel_spmd(nc, in_maps, list(range(N_CORES)))
    outs = np.stack([res.results[j]["out"] for j in range(N_CORES)])

    # ---- un-route ----
    flat = (outs.reshape(N_CORES, B, SEG).transpose(1, 0, 2).reshape(NPAD))
    out_full = np.empty(N, dtype=np.float32)
    out_full[order[ok]] = flat[gslot]

    if not ok.all():  # pathological bucket skew: finish stragglers on host
        rest = order[~ok]
        acc = _hinge_np(mvoc[rest], base_knots, base_w, base_b)
        bi = bidx[rest]
        acc += (np.maximum(mvoc[rest, None] - adj_knots[bi], 0.0)
                * adj_w[bi]).sum(axis=1) + adj_b[bi]
        acc += _hinge_np(lev[rest], idx_knots, idx_w, idx_b)
        acc += _hinge_np(wapv[rest], wap_knots, wap_w, wap_b)
        acc += _hinge_np(cpn[rest], cpn_knots, cpn_w, cpn_b)
        out_full[rest] = (acc + bias).astype(np.float32)

    return out_full


# revision 15
# speedup vs baseline: 4.6234x; 4.6234x over previous
"""Trainium2 Bass kernel for nn_CLOSpreadModel (moe_routing, 8 cores).

Math (per sample n):
  out[n] = hinge(mvoc; base) + hinge(mvoc; adj[bucket_idx[n]]) + adj_b[bucket]
         + hinge(lev; idx) + hinge(wap; wap) + hinge(cpn; cpn) + bias
  where hinge(x; knots,w,b) = sum_k w_k * relu(x - knot_k) + b    (K=64)

Strategy:
  * Host: route samples by bucket (the MoE routing): stable-sort by bucket_idx,
    pad each bucket's segment to a fixed 65536-slot region, deal the slots to
    8 cores so every core gets a fixed [16 buckets x 8192 samples] layout.
    All knot/weight tables are packed host-side into small constant matrices.
  * Device (identical program on all 8 cores, 131072 samples each):
    For each 512-sample chunk, the PE builds delta tiles D[(h,k), f] =
    x_h[f] - knot_hk with tiny-contraction matmuls (rhs rows = [1, mvoc, lev,
    wap, cpn, cpn_shift]); ACT/DVE apply relu PSUM->SBUF; PE contracts the
    128 knot-rows against weight vectors, accumulating all five hinges (plus
    the summed bias via a dedicated ones-row matmul) into per-sample PSUM
    rows.  The two cpn hinges of a chunk pair share one delta tile via a
    512-shifted cpn row, saving a fifth of the PE passes.
"""

import os
import sys

import numpy as np

sys.path.insert(0, "/opt/trn_rl_repo")

N_CORES = 8
B = 16          # buckets
K = 64          # knots per hinge
SEG = 8192      # samples per (core, bucket) block
NCORE = B * SEG           # samples per core = 131072
NPAD = N_CORES * NCORE    # padded total = 1048576
F = 512                   # chunk size (one PSUM bank of fp32)

LHW = 18 * 128            # lhsT blocks: 16x A[b], B, C-pair
WS1 = LHW                 # 17 single-col weight blocks, 15 wide each
W3P = WS1 + 17 * 15       # dual-col cpn weight block, 16 wide
BIA = W3P + 16            # per-bucket bias strip on row 0, 8 wide each
CSTW = BIA + 16 * 8

_CACHE = {}


def _build_bass(nblocks=B):
    """Build the per-core Bass module (identical for all cores)."""
    import concourse.bass as bass
    import concourse.bacc as bacc
    import concourse.mybir as mybir
    from concourse.tile import TileContext
    from contextlib import ExitStack

    f32 = mybir.dt.float32
    Relu = mybir.ActivationFunctionType.Relu

    nc = bacc.Bacc()
    xs = nc.declare_dram_parameter("xs", [6, NCORE], f32, isOutput=False)
    cst = nc.declare_dram_parameter("cst", [128, CSTW], f32, isOutput=False)
    out = nc.declare_dram_parameter("out", [NCORE], f32, isOutput=True)

    with TileContext(nc) as tc:
        with ExitStack() as ctx:
            const_pool = ctx.enter_context(tc.tile_pool(name="const", bufs=1))
            xs_pool = ctx.enter_context(tc.tile_pool(name="xsp", bufs=2))
            r_pool = ctx.enter_context(tc.tile_pool(name="rp", bufs=2))
            osb_pool = ctx.enter_context(tc.tile_pool(name="osb", bufs=2))
            d_pool = ctx.enter_context(tc.tile_pool(name="dp", bufs=2, space="PSUM"))
            o_pool = ctx.enter_context(tc.tile_pool(name="op", bufs=2, space="PSUM"))

            cst_sb = const_pool.tile([128, CSTW], f32)
            nc.sync.dma_start(out=cst_sb[:], in_=cst[:, :])
            lh = cst_sb[0:6, 0:LHW]

            def wv1(k, j):        # [128, 8], weight col k at output row j
                return cst_sb[:, WS1 + k * 15 + 7 - j: WS1 + k * 15 + 15 - j]

            def wv3(t):           # [128, 8], cpn pair cols at rows 2t, 2t+1
                return cst_sb[:, W3P + 7 - 2 * t: W3P + 15 - 2 * t]

            # startup absorber: takes the const-DMA wait on its own ldweights
            # so no later matmul needs two sync waits (walrus limit is one)
            warm = o_pool.tile([8, F], f32, tag="ob")
            nc.tensor.matmul(warm[:, 0:1], cst_sb[0:5, 0:8], cst_sb[0:5, 0:1],
                             start=True, stop=True, skip_group_check=True)

            for b in range(nblocks):
                xs_sb = xs_pool.tile([6, SEG], f32, tag="xs")
                nc.sync.dma_start(out=xs_sb[:], in_=xs[:, b * SEG:(b + 1) * SEG])
                for g in range(SEG // (8 * F)):      # 2 groups of 8 chunks
                    ob = o_pool.tile([8, F], f32, tag="ob")
                    # summed bias for this block, via the ones row
                    c0 = g * 8
                    nc.tensor.matmul(
                        ob[:], cst_sb[0:1, BIA + b * 8: BIA + b * 8 + 8],
                        xs_sb[0:1, c0 * F:(c0 + 1) * F],
                        start=True, stop=False, skip_group_check=True)
                    for t in range(4):               # chunk pairs
                        ca = g * 8 + 2 * t
                        rhs_a = xs_sb[:, ca * F:(ca + 1) * F]
                        rhs_b = xs_sb[:, (ca + 1) * F:(ca + 2) * F]
                        d1a = d_pool.tile([128, F], f32, tag="d1")
                        nc.tensor.matmul(d1a[:], lh[:, b * 128:(b + 1) * 128],
                                         rhs_a, start=True, stop=True)
                        d1b = d_pool.tile([128, F], f32, tag="d1")
                        nc.tensor.matmul(d1b[:], lh[:, b * 128:(b + 1) * 128],
                                         rhs_b, start=True, stop=True)
                        d2a = d_pool.tile([128, F], f32, tag="d2")
                        nc.tensor.matmul(d2a[:], lh[:, 16 * 128:17 * 128],
                                         rhs_a, start=True, stop=True)
                        d2b = d_pool.tile([128, F], f32, tag="d2")
                        nc.tensor.matmul(d2b[:], lh[:, 16 * 128:17 * 128],
                                         rhs_b, start=True, stop=True)
                        d3 = d_pool.tile([128, F], f32, tag="d3")
                        nc.tensor.matmul(d3[:], lh[:, 17 * 128:18 * 128],
                                         rhs_a, start=True, stop=True)

                        r1a = r_pool.tile([128, F], f32, tag="r1")
                        nc.scalar.activation(r1a[:], d1a[:], Relu)
                        r1b = r_pool.tile([128, F], f32, tag="r1b")
                        nc.scalar.activation(r1b[:], d1b[:], Relu)
                        r2a = r_pool.tile([128, F], f32, tag="r2")
                        nc.vector.tensor_scalar_max(r2a[:], d2a[:], 0.0)
                        r2b = r_pool.tile([128, F], f32, tag="r2b")
                        nc.vector.tensor_scalar_max(r2b[:], d2b[:], 0.0)
                        r3 = r_pool.tile([128, F], f32, tag="r3")
                        if t % 2 == 0:
                            nc.vector.tensor_scalar_max(r3[:], d3[:], 0.0)
                        else:
                            nc.scalar.activation(r3[:], d3[:], Relu)

                        nc.tensor.matmul(ob[:], wv1(b, 2 * t), r1a[:],
                                         start=False, stop=False,
                                         skip_group_check=True)
                        nc.tensor.matmul(ob[:], wv1(b, 2 * t + 1), r1b[:],
                                         start=False, stop=False,
                                         skip_group_check=True)
                        nc.tensor.matmul(ob[:], wv1(16, 2 * t), r2a[:],
                                         start=False, stop=False,
                                         skip_group_check=True)
                        nc.tensor.matmul(ob[:], wv1(16, 2 * t + 1), r2b[:],
                                         start=False, stop=False,
                                         skip_group_check=True)
                        nc.tensor.matmul(ob[:], wv3(t), r3[:],
                                         start=False, stop=(t == 3),
                                         skip_group_check=True)
                    out_sb = osb_pool.tile([8, F], f32, tag="out")
                    if g % 2 == 0:
                        nc.vector.tensor_copy(out_sb[:], ob[:])
                    else:
                        nc.scalar.copy(out_sb[:], ob[:])
                    dst = out[b * SEG + g * 8 * F: b * SEG + (g + 1) * 8 * F]
                    nc.sync.dma_start(
                        out=dst.rearrange("(p f) -> p f", p=8), in_=out_sb[:]
                    )
    nc.finalize()
    return nc


def _pack_consts(base_knots, base_w, base_b, adj_knots, adj_w, adj_b,
                 idx_knots, idx_w, idx_b, wap_knots, wap_w, wap_b,
                 cpn_knots, cpn_w, cpn_b, bias):
    """Pack parameters into the single constant matrix (see _build_bass)."""
    cst = np.zeros((128, CSTW), dtype=np.float32)
    for b in range(B):
        c0 = b * 128
        cst[0, c0:c0 + 64] = -base_knots
        cst[1, c0:c0 + 64] = 1.0
        cst[0, c0 + 64:c0 + 128] = -adj_knots[b]
        cst[1, c0 + 64:c0 + 128] = 1.0
    c0 = 16 * 128
    cst[0, c0:c0 + 64] = -idx_knots
    cst[2, c0:c0 + 64] = 1.0
    cst[0, c0 + 64:c0 + 128] = -wap_knots
    cst[3, c0 + 64:c0 + 128] = 1.0
    c0 = 17 * 128          # cpn pair block: row4 = this chunk, row5 = next
    cst[0, c0:c0 + 64] = -cpn_knots
    cst[4, c0:c0 + 64] = 1.0
    cst[0, c0 + 64:c0 + 128] = -cpn_knots
    cst[5, c0 + 64:c0 + 128] = 1.0
    # single-col weight blocks, zero-padded 7 each side
    for b in range(B):
        cst[0:64, WS1 + b * 15 + 7] = base_w
        cst[64:128, WS1 + b * 15 + 7] = adj_w[b]
    cst[0:64, WS1 + 16 * 15 + 7] = idx_w
    cst[64:128, WS1 + 16 * 15 + 7] = wap_w
    # dual-col cpn block
    cst[0:64, W3P + 7] = cpn_w
    cst[64:128, W3P + 8] = cpn_w
    # bias strip (row 0 feeds the ones-row matmul)
    for b in range(B):
        tot = base_b + adj_b[b] + idx_b + wap_b + cpn_b + bias
        cst[0, BIA + b * 8: BIA + b * 8 + 8] = tot
    return cst


def _hinge_np(x, knots, w, b):
    return np.maximum(x[:, None] - knots[None, :], 0.0) @ w + b


def kernel(mvoc, bucket_idx, lev_idx, wap, cpnspread,
           base_knots, base_w, base_b,
           adj_knots, adj_w, adj_b,
           idx_knots, idx_w, idx_b,
           wap_knots, wap_w, wap_b,
           cpn_knots, cpn_w, cpn_b, bias, **_unused):
    from concourse.bass_utils import run_bass_kernel_spmd

    mvoc = np.asarray(mvoc, dtype=np.float32)
    bidx = np.asarray(bucket_idx).astype(np.int64)
    lev = np.asarray(lev_idx, dtype=np.float32)
    wapv = np.asarray(wap, dtype=np.float32)
    cpn = np.asarray(cpnspread, dtype=np.float32)
    N = mvoc.shape[0]

    params = [np.asarray(a, dtype=np.float32) for a in (
        base_knots, base_w, base_b, adj_knots, adj_w, adj_b,
        idx_knots, idx_w, idx_b, wap_knots, wap_w, wap_b,
        cpn_knots, cpn_w, cpn_b, bias)]
    (base_knots, base_w, base_b, adj_knots, adj_w, adj_b,
     idx_knots, idx_w, idx_b, wap_knots, wap_w, wap_b,
     cpn_knots, cpn_w, cpn_b, bias) = params
    adj_knots = adj_knots.reshape(B, K)
    adj_w = adj_w.reshape(B, K)
    base_b = float(base_b); idx_b = float(idx_b); wap_b = float(wap_b)
    cpn_b = float(cpn_b); bias = float(bias)

    # ---- host-side routing: stable sort by bucket, pad to fixed layout ----
    CAP = NPAD // B                       # 65536 slots per bucket
    order = np.argsort(bidx, kind="stable")
    sorted_b = bidx[order]
    counts = np.bincount(bidx, minlength=B)
    starts = np.concatenate([[0], np.cumsum(counts)[:-1]])
    within = np.arange(N, dtype=np.int64) - starts[sorted_b]
    ok = within < CAP                     # overflow guard (never for ~uniform buckets)
    gslot = sorted_b[ok] * CAP + within[ok]

    def route(x):
        xp = np.zeros(NPAD, dtype=np.float32)
        xp[gslot] = x[order][ok]
        # [B, CAP] -> [B, N_CORES, SEG] -> [N_CORES, B, SEG] -> [N_CORES, NCORE]
        return (xp.reshape(B, N_CORES, SEG).transpose(1, 0, 2)
                .reshape(N_CORES, NCORE))

    xs_all = np.ones((N_CORES, 6, NCORE), dtype=np.float32)
    xs_all[:, 1] = route(mvoc)
    xs_all[:, 2] = route(lev)
    xs_all[:, 3] = route(wapv)
    xs_all[:, 4] = route(cpn)
    xs_all[:, 5, :-F] = xs_all[:, 4, F:]   # cpn shifted left one chunk
    xs_all[:, 5, -F:] = 0.0

    cstm = _pack_consts(base_knots, base_w, base_b, adj_knots, adj_w, adj_b,
                        idx_knots, idx_w, idx_b, wap_knots, wap_w, wap_b,
                        cpn_knots, cpn_w, cpn_b, bias)

    if "nc" not in _CACHE:
        _CACHE["nc"] = _build_bass()
    nc = _CACHE["nc"]

    in_maps = [{"xs": xs_all[j], "cst": cstm} for j in range(N_CORES)]
    _CACHE["in_maps"] = in_maps
    res = run_bass_kernel_spmd(nc, in_maps, list(range(N_CORES)))
    outs = np.stack([res.results[j]["out"] for j in range(N_CORES)])

    # ---- un-route ----
    flat = (outs.reshape(N_CORES, B, SEG).transpose(1, 0, 2).reshape(NPAD))
    out_full = np.empty(N, dtype=np.float32)
    out_full[order[ok]] = flat[gslot]

    if not ok.all():  # pathological bucket skew: finish stragglers on host
        rest = order[~ok]
        acc = _hinge_np(mvoc[rest], base_knots, base_w, base_b)
        bi = bidx[rest]
        acc += (np.maximum(mvoc[rest, None] - adj_knots[bi], 0.0)
                * adj_w[bi]).sum(axis=1) + adj_b[bi]
        acc += _hinge_np(lev[rest], idx_knots, idx_w, idx_b)
        acc += _hinge_np(wapv[rest], wap_knots, wap_w, wap_b)
        acc += _hinge_np(cpn[rest], cpn_knots, cpn_w, cpn_b)
        out_full[rest] = (acc + bias).astype(np.float32)

    return out_full


# revision 19
# speedup vs baseline: 10.7604x; 2.3274x over previous
"""Trainium2 Bass kernel for nn_CLOSpreadModel (moe_routing, 8 cores).

Math (per sample n):
  out[n] = hinge(mvoc; base) + hinge(mvoc; adj[bucket_idx[n]]) + adj_b[bucket]
         + hinge(lev; idx) + hinge(wap; wap) + hinge(cpn; cpn) + bias
  where hinge(x; knots,w,b) = sum_k w_k * relu(x - knot_k) + b    (K=64)

Strategy:
  * Host: route samples by bucket (the MoE routing): stable-sort by bucket_idx,
    pad each bucket's segment to a fixed 65536-slot region, deal the slots to
    8 cores so every core gets a fixed [16 buckets x 8192 samples] layout.
    All knot/weight tables are packed host-side into small constant matrices.
  * Device (identical program on all 8 cores, 131072 samples each):
    For each 512-sample chunk, the PE builds delta tiles D[(h,k), f] =
    x_h[f] - knot_hk with tiny-contraction matmuls (rhs rows = [1, mvoc, lev,
    wap, cpn, cpn_shift]); ACT/DVE apply relu PSUM->SBUF; PE contracts the
    128 knot-rows against weight vectors, accumulating all five hinges (plus
    the summed bias via a dedicated ones-row matmul) into per-sample PSUM
    rows.  The two cpn hinges of a chunk pair share one delta tile via a
    512-shifted cpn row, saving a fifth of the PE passes.
"""

import os
import sys

import numpy as np

sys.path.insert(0, "/opt/trn_rl_repo")

N_CORES = 8
B = 16          # buckets
K = 64          # knots per hinge
SEG = 8192      # samples per (core, bucket) block
NCORE = B * SEG           # samples per core = 131072
NPAD = N_CORES * NCORE    # padded total = 1048576
F = 512                   # chunk size (one PSUM bank of fp32)

LHW = 18 * 128            # lhsT blocks: 16x A[b], B, C-pair
WS1 = LHW                 # 17 single-col weight blocks, 15 wide each
W3P = WS1 + 17 * 15       # dual-col cpn weight block, 16 wide
BIA = W3P + 16            # per-bucket bias strip on row 0, 8 wide each
CSTW = BIA + 16 * 8

_CACHE = {}


def _build_bass(nblocks=B):
    """Build the per-core Bass module (identical for all cores)."""
    import concourse.bass as bass
    import concourse.bacc as bacc
    import concourse.mybir as mybir
    from concourse.tile import TileContext
    from contextlib import ExitStack

    f32 = mybir.dt.float32
    f32r = mybir.dt.float32r     # fp32 bytes, single-pass PE (4x faster)
    Relu = mybir.ActivationFunctionType.Relu

    nc = bacc.Bacc()
    xs = nc.declare_dram_parameter("xs", [6, NCORE], f32r, isOutput=False)
    cst = nc.declare_dram_parameter("cst", [128, CSTW], f32r, isOutput=False)
    out = nc.declare_dram_parameter("out", [NCORE], f32, isOutput=True)

    with TileContext(nc) as tc:
        with ExitStack() as ctx:
            const_pool = ctx.enter_context(tc.tile_pool(name="const", bufs=1))
            xs_pool = ctx.enter_context(tc.tile_pool(name="xsp", bufs=2))
            r_pool = ctx.enter_context(tc.tile_pool(name="rp", bufs=2))
            osb_pool = ctx.enter_context(tc.tile_pool(name="osb", bufs=2))
            d_pool = ctx.enter_context(tc.tile_pool(name="dp", bufs=2, space="PSUM"))
            o_pool = ctx.enter_context(tc.tile_pool(name="op", bufs=2, space="PSUM"))

            cst_sb = const_pool.tile([128, CSTW], f32r)
            nc.sync.dma_start(out=cst_sb[:], in_=cst[:, :])
            lh = cst_sb[0:6, 0:LHW]

            def wv1(k, j):        # [128, 8], weight col k at output row j
                return cst_sb[:, WS1 + k * 15 + 7 - j: WS1 + k * 15 + 15 - j]

            def wv3(t):           # [128, 8], cpn pair cols at rows 2t, 2t+1
                return cst_sb[:, W3P + 7 - 2 * t: W3P + 15 - 2 * t]

            # startup absorber: takes the const-DMA wait on its own ldweights
            # so no later matmul needs two sync waits (walrus limit is one)
            warm = o_pool.tile([8, F], f32, tag="ob")
            nc.tensor.matmul(warm[:], cst_sb[0:5, 0:8], cst_sb[0:5, 0:F],
                             start=True, stop=True, skip_group_check=True)

            for b in range(nblocks):
                xs_sb = xs_pool.tile([6, SEG], f32r, tag="xs")
                nc.sync.dma_start(out=xs_sb[:], in_=xs[:, b * SEG:(b + 1) * SEG])
                for g in range(SEG // (8 * F)):      # 2 groups of 8 chunks
                    ob = o_pool.tile([8, F], f32, tag="ob")
                    # summed bias for this block, via the ones row
                    c0 = g * 8
                    nc.tensor.matmul(
                        ob[:], cst_sb[0:1, BIA + b * 8: BIA + b * 8 + 8],
                        xs_sb[0:1, c0 * F:(c0 + 1) * F],
                        start=True, stop=False, skip_group_check=True)
                    for t in range(4):               # chunk pairs
                        ca = g * 8 + 2 * t
                        rhs_a = xs_sb[:, ca * F:(ca + 1) * F]
                        rhs_b = xs_sb[:, (ca + 1) * F:(ca + 2) * F]
                        d1a = d_pool.tile([128, F], f32, tag="d1")
                        nc.tensor.matmul(d1a[:], lh[:, b * 128:(b + 1) * 128],
                                         rhs_a, start=True, stop=True)
                        d1b = d_pool.tile([128, F], f32, tag="d1")
                        nc.tensor.matmul(d1b[:], lh[:, b * 128:(b + 1) * 128],
                                         rhs_b, start=True, stop=True)
                        d2a = d_pool.tile([128, F], f32, tag="d2")
                        nc.tensor.matmul(d2a[:], lh[:, 16 * 128:17 * 128],
                                         rhs_a, start=True, stop=True)
                        d2b = d_pool.tile([128, F], f32, tag="d2")
                        nc.tensor.matmul(d2b[:], lh[:, 16 * 128:17 * 128],
                                         rhs_b, start=True, stop=True)
                        d3 = d_pool.tile([128, F], f32, tag="d3")
                        nc.tensor.matmul(d3[:], lh[:, 17 * 128:18 * 128],
                                         rhs_a, start=True, stop=True)

                        r1a = r_pool.tile([128, F], f32r, tag="r1")
                        nc.scalar.activation(r1a[:], d1a[:], Relu)
                        r1b = r_pool.tile([128, F], f32r, tag="r1b")
                        nc.scalar.activation(r1b[:], d1b[:], Relu)
                        r2a = r_pool.tile([128, F], f32r, tag="r2")
                        nc.vector.tensor_scalar_max(r2a[:], d2a[:], 0.0)
                        r2b = r_pool.tile([128, F], f32r, tag="r2b")
                        nc.vector.tensor_scalar_max(r2b[:], d2b[:], 0.0)
                        r3 = r_pool.tile([128, F], f32r, tag="r3")
                        if t % 2 == 0:
                            nc.vector.tensor_scalar_max(r3[:], d3[:], 0.0)
                        else:
                            nc.scalar.activation(r3[:], d3[:], Relu)

                        nc.tensor.matmul(ob[:], wv1(b, 2 * t), r1a[:],
                                         start=False, stop=False,
                                         skip_group_check=True)
                        nc.tensor.matmul(ob[:], wv1(b, 2 * t + 1), r1b[:],
                                         start=False, stop=False,
                                         skip_group_check=True)
                        nc.tensor.matmul(ob[:], wv1(16, 2 * t), r2a[:],
                                         start=False, stop=False,
                                         skip_group_check=True)
                        nc.tensor.matmul(ob[:], wv1(16, 2 * t + 1), r2b[:],
                                         start=False, stop=False,
                                         skip_group_check=True)
                        nc.tensor.matmul(ob[:], wv3(t), r3[:],
                                         start=False, stop=(t == 3),
                                         skip_group_check=True)
                    out_sb = osb_pool.tile([8, F], f32, tag="out")
                    if g % 2 == 0:
                        nc.vector.tensor_copy(out_sb[:], ob[:])
                    else:
                        nc.scalar.copy(out_sb[:], ob[:])
                    dst = out[b * SEG + g * 8 * F: b * SEG + (g + 1) * 8 * F]
                    nc.sync.dma_start(
                        out=dst.rearrange("(p f) -> p f", p=8), in_=out_sb[:]
                    )
    nc.finalize()
    return nc


def _pack_consts(base_knots, base_w, base_b, adj_knots, adj_w, adj_b,
                 idx_knots, idx_w, idx_b, wap_knots, wap_w, wap_b,
                 cpn_knots, cpn_w, cpn_b, bias):
    """Pack parameters into the single constant matrix (see _build_bass)."""
    cst = np.zeros((128, CSTW), dtype=np.float32)
    for b in range(B):
        c0 = b * 128
        cst[0, c0:c0 + 64] = -base_knots
        cst[1, c0:c0 + 64] = 1.0
        cst[0, c0 + 64:c0 + 128] = -adj_knots[b]
        cst[1, c0 + 64:c0 + 128] = 1.0
    c0 = 16 * 128
    cst[0, c0:c0 + 64] = -idx_knots
    cst[2, c0:c0 + 64] = 1.0
    cst[0, c0 + 64:c0 + 128] = -wap_knots
    cst[3, c0 + 64:c0 + 128] = 1.0
    c0 = 17 * 128          # cpn pair block: row4 = this chunk, row5 = next
    cst[0, c0:c0 + 64] = -cpn_knots
    cst[4, c0:c0 + 64] = 1.0
    cst[0, c0 + 64:c0 + 128] = -cpn_knots
    cst[5, c0 + 64:c0 + 128] = 1.0
    # single-col weight blocks, zero-padded 7 each side
    for b in range(B):
        cst[0:64, WS1 + b * 15 + 7] = base_w
        cst[64:128, WS1 + b * 15 + 7] = adj_w[b]
    cst[0:64, WS1 + 16 * 15 + 7] = idx_w
    cst[64:128, WS1 + 16 * 15 + 7] = wap_w
    # dual-col cpn block
    cst[0:64, W3P + 7] = cpn_w
    cst[64:128, W3P + 8] = cpn_w
    # bias strip (row 0 feeds the ones-row matmul)
    for b in range(B):
        tot = base_b + adj_b[b] + idx_b + wap_b + cpn_b + bias
        cst[0, BIA + b * 8: BIA + b * 8 + 8] = tot
    return cst


def _hinge_np(x, knots, w, b):
    return np.maximum(x[:, None] - knots[None, :], 0.0) @ w + b


def kernel(mvoc, bucket_idx, lev_idx, wap, cpnspread,
           base_knots, base_w, base_b,
           adj_knots, adj_w, adj_b,
           idx_knots, idx_w, idx_b,
           wap_knots, wap_w, wap_b,
           cpn_knots, cpn_w, cpn_b, bias, **_unused):
    from concourse.bass_utils import run_bass_kernel_spmd

    mvoc = np.asarray(mvoc, dtype=np.float32)
    bidx = np.asarray(bucket_idx).astype(np.int64)
    lev = np.asarray(lev_idx, dtype=np.float32)
    wapv = np.asarray(wap, dtype=np.float32)
    cpn = np.asarray(cpnspread, dtype=np.float32)
    N = mvoc.shape[0]

    params = [np.asarray(a, dtype=np.float32) for a in (
        base_knots, base_w, base_b, adj_knots, adj_w, adj_b,
        idx_knots, idx_w, idx_b, wap_knots, wap_w, wap_b,
        cpn_knots, cpn_w, cpn_b, bias)]
    (base_knots, base_w, base_b, adj_knots, adj_w, adj_b,
     idx_knots, idx_w, idx_b, wap_knots, wap_w, wap_b,
     cpn_knots, cpn_w, cpn_b, bias) = params
    adj_knots = adj_knots.reshape(B, K)
    adj_w = adj_w.reshape(B, K)
    base_b = float(base_b); idx_b = float(idx_b); wap_b = float(wap_b)
    cpn_b = float(cpn_b); bias = float(bias)

    # ---- host-side routing: stable sort by bucket, pad to fixed layout ----
    CAP = NPAD // B                       # 65536 slots per bucket
    order = np.argsort(bidx, kind="stable")
    sorted_b = bidx[order]
    counts = np.bincount(bidx, minlength=B)
    starts = np.concatenate([[0], np.cumsum(counts)[:-1]])
    within = np.arange(N, dtype=np.int64) - starts[sorted_b]
    ok = within < CAP                     # overflow guard (never for ~uniform buckets)
    gslot = sorted_b[ok] * CAP + within[ok]

    def route(x):
        xp = np.zeros(NPAD, dtype=np.float32)
        xp[gslot] = x[order][ok]
        # [B, CAP] -> [B, N_CORES, SEG] -> [N_CORES, B, SEG] -> [N_CORES, NCORE]
        return (xp.reshape(B, N_CORES, SEG).transpose(1, 0, 2)
                .reshape(N_CORES, NCORE))

    xs_all = np.ones((N_CORES, 6, NCORE), dtype=np.float32)
    xs_all[:, 1] = route(mvoc)
    xs_all[:, 2] = route(lev)
    xs_all[:, 3] = route(wapv)
    xs_all[:, 4] = route(cpn)
    xs_all[:, 5, :-F] = xs_all[:, 4, F:]   # cpn shifted left one chunk
    xs_all[:, 5, -F:] = 0.0

    cstm = _pack_consts(base_knots, base_w, base_b, adj_knots, adj_w, adj_b,
                        idx_knots, idx_w, idx_b, wap_knots, wap_w, wap_b,
                        cpn_knots, cpn_w, cpn_b, bias)

    if "nc" not in _CACHE:
        _CACHE["nc"] = _build_bass()
    nc = _CACHE["nc"]

    in_maps = [{"xs": xs_all[j], "cst": cstm} for j in range(N_CORES)]
    _CACHE["in_maps"] = in_maps
    res = run_bass_kernel_spmd(nc, in_maps, list(range(N_CORES)))
    outs = np.stack([res.results[j]["out"] for j in range(N_CORES)])

    # ---- un-route ----
    flat = (outs.reshape(N_CORES, B, SEG).transpose(1, 0, 2).reshape(NPAD))
    out_full = np.empty(N, dtype=np.float32)
    out_full[order[ok]] = flat[gslot]

    if not ok.all():  # pathological bucket skew: finish stragglers on host
        rest = order[~ok]
        acc = _hinge_np(mvoc[rest], base_knots, base_w, base_b)
        bi = bidx[rest]
        acc += (np.maximum(mvoc[rest, None] - adj_knots[bi], 0.0)
                * adj_w[bi]).sum(axis=1) + adj_b[bi]
        acc += _hinge_np(lev[rest], idx_knots, idx_w, idx_b)
        acc += _hinge_np(wapv[rest], wap_knots, wap_w, wap_b)
        acc += _hinge_np(cpn[rest], cpn_knots, cpn_w, cpn_b)
        out_full[rest] = (acc + bias).astype(np.float32)

    return out_full


# revision 23
# speedup vs baseline: 31.6054x; 2.9372x over previous
"""Trainium2 Bass kernel for nn_CLOSpreadModel (moe_routing, 8 cores).

Math (per sample n):
  out[n] = hinge(mvoc; base) + hinge(mvoc; adj[bucket_idx[n]]) + adj_b[bucket]
         + hinge(lev; idx) + hinge(wap; wap) + hinge(cpn; cpn) + bias
  where hinge(x; knots,w,b) = sum_k w_k * relu(x - knot_k) + b    (K=64)

Strategy:
  * Host: route samples by bucket (the MoE routing): stable-sort by bucket_idx,
    pad each bucket's segment to a fixed 65536-slot region, deal the slots to
    8 cores so every core gets a fixed [16 buckets x 8192 samples] layout.
    All knot/weight tables are packed host-side into small constant matrices.
  * Device (identical program on all 8 cores, 131072 samples each):
    For each 512-sample chunk, the PE builds delta tiles D[(h,k), f] =
    x_h[f] - knot_hk with tiny-contraction matmuls (rhs rows = [1, mvoc, lev,
    wap, cpn, cpn_shift]); ACT/DVE apply relu PSUM->SBUF; PE contracts the
    128 knot-rows against weight vectors, accumulating all five hinges (plus
    the summed bias via a dedicated ones-row matmul) into per-sample PSUM
    rows.  The two cpn hinges of a chunk pair share one delta tile via a
    512-shifted cpn row, saving a fifth of the PE passes.
"""

import os
import sys

import numpy as np

sys.path.insert(0, "/opt/trn_rl_repo")

N_CORES = 8
B = 16          # buckets
K = 64          # knots per hinge
SEG = 8192      # samples per (core, bucket) block
NCORE = B * SEG           # samples per core = 131072
NPAD = N_CORES * NCORE    # padded total = 1048576
F = 512                   # chunk size (one PSUM bank of fp32)

LHW = 18 * 128            # lhsT blocks: 16x A[b], B, C-pair
WS1 = LHW                 # 17 single-col weight blocks, 15 wide each
W3P = WS1 + 17 * 15       # dual-col cpn weight block, 16 wide
BIA = W3P + 16            # per-bucket bias strip on row 0, 8 wide each
CSTW = BIA + 16 * 8

_CACHE = {}
RP_BUFS = 2
XS_BUFS = 2
OSB_BUFS = 2


def _build_bass(nblocks=B):
    """Build the per-core Bass module (identical for all cores)."""
    import concourse.bass as bass
    import concourse.bacc as bacc
    import concourse.mybir as mybir
    from concourse.tile import TileContext
    from contextlib import ExitStack

    f32 = mybir.dt.float32
    f32r = mybir.dt.float32r     # fp32 bytes, single-pass PE (4x faster)
    Relu = mybir.ActivationFunctionType.Relu

    nc = bacc.Bacc()
    xs = nc.declare_dram_parameter("xs", [6, NCORE], f32r, isOutput=False)
    cst = nc.declare_dram_parameter("cst", [128, CSTW], f32r, isOutput=False)
    out = nc.declare_dram_parameter("out", [NCORE], f32, isOutput=True)

    with TileContext(nc) as tc:
        with ExitStack() as ctx:
            const_pool = ctx.enter_context(tc.tile_pool(name="const", bufs=1))
            xs_pool = ctx.enter_context(tc.tile_pool(name="xsp", bufs=XS_BUFS))
            r_pool = ctx.enter_context(tc.tile_pool(name="rp", bufs=RP_BUFS))
            osb_pool = ctx.enter_context(tc.tile_pool(name="osb", bufs=OSB_BUFS))
            d_pool = ctx.enter_context(tc.tile_pool(name="dp", bufs=2, space="PSUM"))
            o_pool = ctx.enter_context(tc.tile_pool(name="op", bufs=2, space="PSUM"))

            cst_sb = const_pool.tile([128, CSTW], f32r)
            nc.sync.dma_start(out=cst_sb[:], in_=cst[:, :])
            lh = cst_sb[0:6, 0:LHW]

            def wv1(k, j):        # [128, 8], weight col k at output row j
                return cst_sb[:, WS1 + k * 15 + 7 - j: WS1 + k * 15 + 15 - j]

            def wv3(t):           # [128, 8], cpn pair cols at rows 2t, 2t+1
                return cst_sb[:, W3P + 7 - 2 * t: W3P + 15 - 2 * t]

            # startup absorber: takes the const-DMA wait on its own ldweights
            # so no later matmul needs two sync waits (walrus limit is one)
            warm = o_pool.tile([8, F], f32, tag="ob")
            nc.tensor.matmul(warm[:], cst_sb[0:5, 0:8], cst_sb[0:5, 0:F],
                             start=True, stop=True, skip_group_check=True)

            for b in range(nblocks):
                xs_sb = xs_pool.tile([6, SEG], f32r, tag="xs")
                nc.sync.dma_start(out=xs_sb[:], in_=xs[:, b * SEG:(b + 1) * SEG])
                for g in range(SEG // (8 * F)):      # 2 groups of 8 chunks
                    ob = o_pool.tile([8, F], f32, tag="ob")
                    # summed bias for this block, via the ones row
                    c0 = g * 8
                    nc.tensor.matmul(
                        ob[:], cst_sb[0:1, BIA + b * 8: BIA + b * 8 + 8],
                        xs_sb[0:1, c0 * F:(c0 + 1) * F],
                        start=True, stop=False, skip_group_check=True)
                    for t in range(4):               # chunk pairs
                        ca = g * 8 + 2 * t
                        rhs_a = xs_sb[:, ca * F:(ca + 1) * F]
                        rhs_b = xs_sb[:, (ca + 1) * F:(ca + 2) * F]
                        d1a = d_pool.tile([128, F], f32, tag="d1")
                        nc.tensor.matmul(d1a[:], lh[:, b * 128:(b + 1) * 128],
                                         rhs_a, start=True, stop=True)
                        d1b = d_pool.tile([128, F], f32, tag="d1")
                        nc.tensor.matmul(d1b[:], lh[:, b * 128:(b + 1) * 128],
                                         rhs_b, start=True, stop=True)
                        d2a = d_pool.tile([128, F], f32, tag="d2")
                        nc.tensor.matmul(d2a[:], lh[:, 16 * 128:17 * 128],
                                         rhs_a, start=True, stop=True)
                        d2b = d_pool.tile([128, F], f32, tag="d2")
                        nc.tensor.matmul(d2b[:], lh[:, 16 * 128:17 * 128],
                                         rhs_b, start=True, stop=True)
                        d3 = d_pool.tile([128, F], f32, tag="d3")
                        nc.tensor.matmul(d3[:], lh[:, 17 * 128:18 * 128],
                                         rhs_a, start=True, stop=True)

                        r1a = r_pool.tile([128, F], f32r, tag="r1")
                        nc.scalar.activation(r1a[:], d1a[:], Relu)
                        r1b = r_pool.tile([128, F], f32r, tag="r1b")
                        nc.scalar.activation(r1b[:], d1b[:], Relu)
                        r2a = r_pool.tile([128, F], f32r, tag="r2")
                        nc.vector.tensor_scalar_max(r2a[:], d2a[:], 0.0)
                        r2b = r_pool.tile([128, F], f32r, tag="r2b")
                        nc.vector.tensor_scalar_max(r2b[:], d2b[:], 0.0)
                        r3 = r_pool.tile([128, F], f32r, tag="r3")
                        if t % 2 == 0:
                            nc.vector.tensor_scalar_max(r3[:], d3[:], 0.0)
                        else:
                            nc.scalar.activation(r3[:], d3[:], Relu)

                        nc.tensor.matmul(ob[:], wv1(b, 2 * t), r1a[:],
                                         start=False, stop=False,
                                         skip_group_check=True)
                        nc.tensor.matmul(ob[:], wv1(b, 2 * t + 1), r1b[:],
                                         start=False, stop=False,
                                         skip_group_check=True)
                        nc.tensor.matmul(ob[:], wv1(16, 2 * t), r2a[:],
                                         start=False, stop=False,
                                         skip_group_check=True)
                        nc.tensor.matmul(ob[:], wv1(16, 2 * t + 1), r2b[:],
                                         start=False, stop=False,
                                         skip_group_check=True)
                        nc.tensor.matmul(ob[:], wv3(t), r3[:],
                                         start=False, stop=(t == 3),
                                         skip_group_check=True)
                    out_sb = osb_pool.tile([8, F], f32, tag="out")
                    if g % 2 == 0:
                        nc.vector.tensor_copy(out_sb[:], ob[:])
                    else:
                        nc.scalar.copy(out_sb[:], ob[:])
                    dst = out[b * SEG + g * 8 * F: b * SEG + (g + 1) * 8 * F]
                    nc.sync.dma_start(
                        out=dst.rearrange("(p f) -> p f", p=8), in_=out_sb[:]
                    )
    nc.finalize()
    return nc


def _pack_consts(base_knots, base_w, base_b, adj_knots, adj_w, adj_b,
                 idx_knots, idx_w, idx_b, wap_knots, wap_w, wap_b,
                 cpn_knots, cpn_w, cpn_b, bias):
    """Pack parameters into the single constant matrix (see _build_bass)."""
    cst = np.zeros((128, CSTW), dtype=np.float32)
    for b in range(B):
        c0 = b * 128
        cst[0, c0:c0 + 64] = -base_knots
        cst[1, c0:c0 + 64] = 1.0
        cst[0, c0 + 64:c0 + 128] = -adj_knots[b]
        cst[1, c0 + 64:c0 + 128] = 1.0
    c0 = 16 * 128
    cst[0, c0:c0 + 64] = -idx_knots
    cst[2, c0:c0 + 64] = 1.0
    cst[0, c0 + 64:c0 + 128] = -wap_knots
    cst[3, c0 + 64:c0 + 128] = 1.0
    c0 = 17 * 128          # cpn pair block: row4 = this chunk, row5 = next
    cst[0, c0:c0 + 64] = -cpn_knots
    cst[4, c0:c0 + 64] = 1.0
    cst[0, c0 + 64:c0 + 128] = -cpn_knots
    cst[5, c0 + 64:c0 + 128] = 1.0
    # single-col weight blocks, zero-padded 7 each side
    for b in range(B):
        cst[0:64, WS1 + b * 15 + 7] = base_w
        cst[64:128, WS1 + b * 15 + 7] = adj_w[b]
    cst[0:64, WS1 + 16 * 15 + 7] = idx_w
    cst[64:128, WS1 + 16 * 15 + 7] = wap_w
    # dual-col cpn block
    cst[0:64, W3P + 7] = cpn_w
    cst[64:128, W3P + 8] = cpn_w
    # bias strip (row 0 feeds the ones-row matmul)
    for b in range(B):
        tot = base_b + adj_b[b] + idx_b + wap_b + cpn_b + bias
        cst[0, BIA + b * 8: BIA + b * 8 + 8] = tot
    return cst


def _hinge_np(x, knots, w, b):
    return np.maximum(x[:, None] - knots[None, :], 0.0) @ w + b


def kernel(mvoc, bucket_idx, lev_idx, wap, cpnspread,
           base_knots, base_w, base_b,
           adj_knots, adj_w, adj_b,
           idx_knots, idx_w, idx_b,
           wap_knots, wap_w, wap_b,
           cpn_knots, cpn_w, cpn_b, bias, **_unused):
    from concourse.bass_utils import run_bass_kernel_spmd

    mvoc = np.asarray(mvoc, dtype=np.float32)
    bidx = np.asarray(bucket_idx).astype(np.int64)
    lev = np.asarray(lev_idx, dtype=np.float32)
    wapv = np.asarray(wap, dtype=np.float32)
    cpn = np.asarray(cpnspread, dtype=np.float32)
    N = mvoc.shape[0]

    params = [np.asarray(a, dtype=np.float32) for a in (
        base_knots, base_w, base_b, adj_knots, adj_w, adj_b,
        idx_knots, idx_w, idx_b, wap_knots, wap_w, wap_b,
        cpn_knots, cpn_w, cpn_b, bias)]
    (base_knots, base_w, base_b, adj_knots, adj_w, adj_b,
     idx_knots, idx_w, idx_b, wap_knots, wap_w, wap_b,
     cpn_knots, cpn_w, cpn_b, bias) = params
    adj_knots = adj_knots.reshape(B, K)
    adj_w = adj_w.reshape(B, K)
    base_b = float(base_b); idx_b = float(idx_b); wap_b = float(wap_b)
    cpn_b = float(cpn_b); bias = float(bias)

    # ---- host-side routing: stable sort by bucket, pad to fixed layout ----
    CAP = NPAD // B                       # 65536 slots per bucket
    order = np.argsort(bidx, kind="stable")
    sorted_b = bidx[order]
    counts = np.bincount(bidx, minlength=B)
    starts = np.concatenate([[0], np.cumsum(counts)[:-1]])
    within = np.arange(N, dtype=np.int64) - starts[sorted_b]
    ok = within < CAP                     # overflow guard (never for ~uniform buckets)
    gslot = sorted_b[ok] * CAP + within[ok]

    def route(x):
        xp = np.zeros(NPAD, dtype=np.float32)
        xp[gslot] = x[order][ok]
        # [B, CAP] -> [B, N_CORES, SEG] -> [N_CORES, B, SEG] -> [N_CORES, NCORE]
        return (xp.reshape(B, N_CORES, SEG).transpose(1, 0, 2)
                .reshape(N_CORES, NCORE))

    xs_all = np.ones((N_CORES, 6, NCORE), dtype=np.float32)
    xs_all[:, 1] = route(mvoc)
    xs_all[:, 2] = route(lev)
    xs_all[:, 3] = route(wapv)
    xs_all[:, 4] = route(cpn)
    xs_all[:, 5, :-F] = xs_all[:, 4, F:]   # cpn shifted left one chunk
    xs_all[:, 5, -F:] = 0.0

    cstm = _pack_consts(base_knots, base_w, base_b, adj_knots, adj_w, adj_b,
                        idx_knots, idx_w, idx_b, wap_knots, wap_w, wap_b,
                        cpn_knots, cpn_w, cpn_b, bias)

    if "nc" not in _CACHE:
        _CACHE["nc"] = _build_bass()
    nc = _CACHE["nc"]

    in_maps = [{"xs": xs_all[j], "cst": cstm} for j in range(N_CORES)]
    _CACHE["in_maps"] = in_maps
    res = run_bass_kernel_spmd(nc, in_maps, list(range(N_CORES)))
    outs = np.stack([res.results[j]["out"] for j in range(N_CORES)])

    # ---- un-route ----
    flat = (outs.reshape(N_CORES, B, SEG).transpose(1, 0, 2).reshape(NPAD))
    out_full = np.empty(N, dtype=np.float32)
    out_full[order[ok]] = flat[gslot]

    if not ok.all():  # pathological bucket skew: finish stragglers on host
        rest = order[~ok]
        acc = _hinge_np(mvoc[rest], base_knots, base_w, base_b)
        bi = bidx[rest]
        acc += (np.maximum(mvoc[rest, None] - adj_knots[bi], 0.0)
                * adj_w[bi]).sum(axis=1) + adj_b[bi]
        acc += _hinge_np(lev[rest], idx_knots, idx_w, idx_b)
        acc += _hinge_np(wapv[rest], wap_knots, wap_w, wap_b)
        acc += _hinge_np(cpn[rest], cpn_knots, cpn_w, cpn_b)
        out_full[rest] = (acc + bias).astype(np.float32)

    return out_full
